# revision 3
# baseline (speedup 1.0000x reference)
"""CompensatedSparseLinear on 8 TRN2 NeuronCores.

out[b,s,o] = sum_i x[b,s,i] * (W[o,i] + delta[o,i]) + b[o]

The sparse COO delta is folded into W on the host (scatter-add), leaving a
dense matmul: out2d = x2d @ W_eff^T + b with x2d [8192, 4096], W_eff [4096, 4096].

Sharding: data-parallel along the 8192 batch*seq rows — 1024 rows per core,
W_eff/b replicated. No collectives; host concatenates the output shards.

Per-core device kernel (out^T layout — out_features on PSUM partitions):
  out^T[n0:n0+128, :] = sum_k W_tile[k, n]^T-block @ x^T[k, m-block] + bias
  - x^T shard [4096, 1024] fp32 resident in SBUF (128 KB/partition)
  - W_eff^T streamed from HBM as pre-tiled [32n, 32k, 128, 128] blocks
  - matmuls in float32r (1 cyc/row vs fp32's 4) accumulated in fp32 PSUM
  - bias added via ScalarE activation(Copy, bias) — per-partition bias
"""

import os
import numpy as np

import concourse.bacc as bacc
import concourse.tile as tile
import concourse.mybir as mybir
from concourse.bass_utils import run_bass_kernel_spmd

# Problem shape (hardcoded — harness contract)
B, S, D_IN, D_OUT = 4, 2048, 4096, 4096
N_CORES = 8
M_TOT = B * S              # 8192 rows
M = M_TOT // N_CORES       # 1024 rows per core
KT = D_IN // 128           # 32 k tiles
NT = D_OUT // 128          # 32 n tiles
MB = 512                   # moving free-dim per matmul (fp32 max)
NMB = M // MB              # 2 m-blocks per core

_MM_DT = mybir.dt.float32r  # matmul operand dtype (4x faster than float32)

_CACHE: dict = {}
LAST = {"exec_time_ns": None}


def _build():
    nc = bacc.Bacc("TRN2", target_bir_lowering=False, debug=False)

    xT = nc.declare_dram_parameter("xT", [KT, 128, M], _MM_DT, isOutput=False)
    wt = nc.declare_dram_parameter("wt", [NT, KT, 128, 128], _MM_DT, isOutput=False)
    bias = nc.declare_dram_parameter("bias", [NT, 128, 1], mybir.dt.float32, isOutput=False)
    outT = nc.declare_dram_parameter("outT", [NT, 128, M], mybir.dt.float32, isOutput=True)

    with tile.TileContext(nc) as tc:
        with (
            tc.tile_pool(name="xp", bufs=1) as xp,
            tc.tile_pool(name="bp", bufs=1) as bp,
            tc.tile_pool(name="wp", bufs=16) as wp,
            tc.tile_pool(name="ps", bufs=8, space="PSUM") as ps,
            tc.tile_pool(name="op", bufs=6) as op,
        ):
            # resident x^T shard: [128, KT*M] = 128 KB/partition
            xs = xp.tile([128, KT * M], _MM_DT)
            xr = xs[:].rearrange("p (k m) -> k p m", k=KT)
            for k in range(KT):
                nc.sync.dma_start(xr[k], xT[k])

            # all bias tiles: [128, NT]
            bs = bp.tile([128, NT], mybir.dt.float32)
            for nt in range(NT):
                nc.sync.dma_start(bs[:, nt : nt + 1], bias[nt])

            for nt in range(NT):
                accs = [
                    ps.tile([128, MB], mybir.dt.float32, tag="ps", name=f"acc_{nt}_{mb}")
                    for mb in range(NMB)
                ]
                for k in range(KT):
                    w = wp.tile([128, 128], _MM_DT, tag="w")
                    nc.sync.dma_start(w[:], wt[nt, k])
                    for mb in range(NMB):
                        nc.tensor.matmul(
                            accs[mb][:],
                            w[:],
                            xr[k][:, mb * MB : (mb + 1) * MB],
                            start=(k == 0),
                            stop=(k == KT - 1),
                        )
                for mb in range(NMB):
                    o = op.tile([128, MB], mybir.dt.float32, tag="o")
                    nc.scalar.activation(
                        o[:],
                        accs[mb][:],
                        mybir.ActivationFunctionType.Identity,
                        bias=bs[:, nt : nt + 1],
                    )
                    nc.sync.dma_start(outT[nt][:, mb * MB : (mb + 1) * MB], o[:])

    nc.compile()
    return nc


def kernel(x, W, b, delta_vals, delta_rows, delta_cols):
    x = np.asarray(x, dtype=np.float32)
    W = np.asarray(W, dtype=np.float32)
    b = np.asarray(b, dtype=np.float32)

    # Fold sparse delta into W (duplicate coords sum)
    W_eff = W.copy()
    np.add.at(W_eff, (np.asarray(delta_rows), np.asarray(delta_cols)), np.asarray(delta_vals, dtype=np.float32))

    # Pre-tile W_eff^T: w_tiles[nt, k, ki, ni] = W_eff[nt*128+ni, k*128+ki]
    w_tiles = np.ascontiguousarray(
        W_eff.reshape(NT, 128, KT, 128).transpose(0, 2, 3, 1)
    )
    bias_t = np.ascontiguousarray(b.reshape(NT, 128, 1))

    x2d = x.reshape(M_TOT, D_IN)
    in_maps = []
    for c in range(N_CORES):
        shard = x2d[c * M : (c + 1) * M]                       # [M, D_IN]
        xT_c = np.ascontiguousarray(shard.T).reshape(KT, 128, M)
        in_maps.append({"xT": xT_c, "wt": w_tiles, "bias": bias_t})

    if "nc" not in _CACHE:
        _CACHE["nc"] = _build()
    nc = _CACHE["nc"]

    trace = os.environ.get("KERNEL_TRACE", "") == "1"
    res = run_bass_kernel_spmd(nc, in_maps, list(range(N_CORES)), trace=trace)
    LAST["exec_time_ns"] = res.exec_time_ns

    out2d = np.empty((M_TOT, D_OUT), dtype=np.float32)
    for c in range(N_CORES):
        outT_c = res.results[c]["outT"].reshape(D_OUT, M)      # [4096, 1024]
        out2d[c * M : (c + 1) * M] = outT_c.T
    return out2d.reshape(B, S, D_OUT)
